# revision 1
# baseline (speedup 1.0000x reference)
"""Bidirectional Keras GRU (reset_after) on 8 Trainium2 NeuronCores.

B=64, T=512, D=H=768. SPMD: identical program on all 8 cores; core c<4 runs the
forward direction for batches 16c:16c+16, core c>=4 runs backward (inputs
pre-reversed in t on host, outputs flipped back). No collectives.

Per core:
  Phase A: input projection xp = x @ W_perm + bfold as M=128 GEMMs
           (stationary = x^T tile over (8 timesteps x 16 batches)), staged to HBM.
  Phase B: T-step recurrence. h^T is the matmul *stationary* operand
           (lhsT [128, 16]), U_perm the moving operand at float32r
           (1 cycle/col, N=384 chunks), split across two PE column groups
           (tile_position (0,0)/(0,64)) so both halves of the 3H gate dim
           stream concurrently. Gates are computed batch-major on DVE/ACT;
           h_new is transposed back to h^T with 6 small PE transposes.
"""

import os
import numpy as np

import concourse.bass as bass
import concourse.bacc as bacc
import concourse.mybir as mybir
import concourse.tile as tile
from concourse.bass_utils import run_bass_kernel_spmd

B, D, H = 64, 768, 768
T = int(os.environ.get("GRU_T", "512"))
G3 = 3 * H          # 2304
NCORE = 8
BL = 16             # batches per core
HH = H // 2         # 384, features per column group
KT = 6              # k-tiles over H/D
NCH = G3 // 384     # 6 column chunks of 384

f32 = mybir.dt.float32
f32r = mybir.dt.float32r
bf16 = mybir.dt.bfloat16
AF = mybir.ActivationFunctionType
OP = mybir.AluOpType

_PERM = np.concatenate([
    np.arange(0, 384), np.arange(768, 1152), np.arange(1536, 1920),      # zA rA hA
    np.arange(384, 768), np.arange(1152, 1536), np.arange(1920, 2304),   # zB rB hB
])
_ZR_MASK = np.zeros(G3, np.float32)
_ZR_MASK[0:1536] = 1.0  # z and r gates in ORIGINAL column space; h gets b1 on-device
# within each 1152-half: z at 0:384, r at 384:768, h at 768:1152
_OFF = {"z": 0, "r": 384, "h": 768}


def build_program(t_steps=T):
    tc_n = t_steps // 8
    nc = bacc.Bacc("TRN2", target_bir_lowering=False, debug=False, num_devices=NCORE)

    x_prep = nc.dram_tensor("x_prep", (KT, tc_n, 128, 128), f32r, kind="ExternalInput")
    w_all = nc.dram_tensor("w_all", (KT, 128, G3), f32r, kind="ExternalInput")
    u_all = nc.dram_tensor("u_all", (KT, 128, G3), bf16, kind="ExternalInput")
    bfold_d = nc.dram_tensor("bfold", (128, G3), f32, kind="ExternalInput")
    b1h_d = nc.dram_tensor("b1h", (128, HH), f32, kind="ExternalInput")
    ident_d = nc.dram_tensor("ident", (128, 16), f32, kind="ExternalInput")
    xp_dram = nc.dram_tensor("xp_scratch", (tc_n, 128, G3), f32, kind="Internal")
    out_core = nc.dram_tensor("out_core", (t_steps, BL, H), f32, kind="ExternalOutput")

    with tile.TileContext(nc) as tctx:
        with (
            tctx.tile_pool(name="const", bufs=1) as cpool,
            tctx.tile_pool(name="xt", bufs=12) as xtpool,
            tctx.tile_pool(name="xps", bufs=3) as xpspool,
            tctx.tile_pool(name="ring", bufs=4) as ringpool,
            tctx.tile_pool(name="hs", bufs=3) as hpool,
            tctx.tile_pool(name="hT", bufs=2) as hTpool,
            tctx.tile_pool(name="tmp", bufs=2) as tmppool,
            tctx.tile_pool(name="act", bufs=2) as actpool,
            tctx.tile_pool(name="psg", bufs=1, space="PSUM") as psg,
            tctx.tile_pool(name="psx", bufs=2, space="PSUM") as psx,
            tctx.tile_pool(name="pst", bufs=2, space="PSUM") as pst,
        ):
            u_sb = cpool.tile([128, KT, G3], bf16, tag="u")
            w_sb = cpool.tile([128, KT, G3], f32r, tag="w")
            bfold = cpool.tile([128, G3], f32, tag="bf")
            b1h = cpool.tile([128, HH], f32, tag="b1h")
            ident = cpool.tile([128, 16], f32, tag="id")

            nc.sync.dma_start(u_sb[:], u_all[:].rearrange("k p c -> p k c"))
            nc.sync.dma_start(w_sb[:], w_all[:].rearrange("k p c -> p k c"))
            nc.sync.dma_start(bfold[:], bfold_d[:])
            nc.sync.dma_start(b1h[:], b1h_d[:])
            nc.sync.dma_start(ident[:], ident_d[:])

            # ---------------- Phase A: input projections -> xp_dram ----------
            for tci in range(tc_n):
                xts = []
                for k in range(KT):
                    xt = xtpool.tile([128, 128], f32r, tag="xt")
                    nc.sync.dma_start(xt[:], x_prep[k, tci])
                    xts.append(xt)
                for c in range(NCH):
                    ps = psx.tile([128, 384], f32, tag="psx")
                    for k in range(KT):
                        nc.tensor.matmul(
                            ps[:], xts[k][:], w_sb[:, k, 384 * c:384 * c + 384],
                            start=(k == 0), stop=(k == KT - 1),
                        )
                    xpc = xpspool.tile([128, 384], f32, tag="xpc")
                    nc.vector.tensor_tensor(
                        xpc[:], ps[:], bfold[:, 384 * c:384 * c + 384], op=OP.add,
                    )
                    nc.gpsimd.dma_start(xp_dram[tci, :, 384 * c:384 * c + 384], xpc[:])

            # ---------------- Phase B: recurrence ----------------------------
            h_prev = hpool.tile([128, HH], f32, tag="h")
            nc.vector.memset(h_prev[:], 0.0)
            hT_cur = hTpool.tile([128, KT, 16], bf16, tag="hT")
            nc.vector.memset(hT_cur[:], 0.0)

            for t in range(t_steps):
                tci, dt = t // 8, t % 8
                ring = ringpool.tile([128, 1152], f32, tag="ring")
                nc.sync.dma_start(ring[0:16, :], xp_dram[tci, 16 * dt:16 * dt + 16, 0:1152])
                nc.sync.dma_start(ring[64:80, :], xp_dram[tci, 16 * dt:16 * dt + 16, 1152:2304])

                ps_gate = {}
                for gate in ("r", "h", "z"):
                    ps = psg.tile([128, 384], f32, tag="ps" + gate)
                    ps_gate[gate] = ps
                    off = _OFF[gate]
                    for k in range(KT):
                        for grp in (0, 1):
                            nc.tensor.matmul(
                                ps[64 * grp:64 * grp + 16, :],
                                hT_cur[:, k, :],
                                u_sb[:, k, 1152 * grp + off:1152 * grp + off + 384],
                                start=(k == 0), stop=(k == KT - 1),
                                tile_position=(0, 64 * grp),
                            )

                rpre = tmppool.tile([128, HH], f32, tag="rpre")
                nc.vector.tensor_tensor(rpre[:], ps_gate["r"][:], ring[:, 384:768], op=OP.add)
                r_t = actpool.tile([128, HH], f32, tag="r")
                nc.scalar.activation(r_t[:], rpre[:], AF.Sigmoid)

                t1 = tmppool.tile([128, HH], f32, tag="t1")
                nc.vector.tensor_tensor(t1[:], ps_gate["h"][:], b1h[:], op=OP.add)
                t2 = tmppool.tile([128, HH], f32, tag="t2")
                nc.vector.tensor_mul(t2[:], r_t[:], t1[:])
                t3 = tmppool.tile([128, HH], f32, tag="t3")
                nc.vector.tensor_tensor(t3[:], t2[:], ring[:, 768:1152], op=OP.add)
                hh = actpool.tile([128, HH], f32, tag="hh")
                nc.scalar.activation(hh[:], t3[:], AF.Tanh)

                zpre = tmppool.tile([128, HH], f32, tag="zpre")
                nc.vector.tensor_tensor(zpre[:], ps_gate["z"][:], ring[:, 0:384], op=OP.add)
                z_t = actpool.tile([128, HH], f32, tag="z")
                nc.scalar.activation(z_t[:], zpre[:], AF.Sigmoid)

                d_t = tmppool.tile([128, HH], f32, tag="d")
                nc.vector.tensor_sub(d_t[:], h_prev[:], hh[:])
                e_t = tmppool.tile([128, HH], f32, tag="e")
                nc.vector.tensor_mul(e_t[:], z_t[:], d_t[:])
                h_new = hpool.tile([128, HH], f32, tag="h")
                nc.vector.tensor_add(h_new[:], e_t[:], hh[:])

                hT_new = hTpool.tile([128, KT, 16], bf16, tag="hT")
                tps = [pst.tile([128, 48], f32, tag="tp", name=f"tp{t}_{i}")
                       for i in range(2)]
                for kt in range(KT):
                    grp, c = kt // 3, kt % 3
                    tp = tps[kt % 2]
                    nc.tensor.transpose(
                        tp[:, (kt // 2) * 16:(kt // 2) * 16 + 16],
                        h_new[64 * grp:64 * grp + 16, 128 * c:128 * c + 128],
                        ident[64 * grp:64 * grp + 16, :],
                        tile_position=(64 * grp, 0),
                    )
                    nc.vector.tensor_copy(hT_new[:, kt, :], tp[:, (kt // 2) * 16:(kt // 2) * 16 + 16])

                nc.gpsimd.dma_start(out_core[t, :, 0:HH], h_new[0:16, :])
                nc.gpsimd.dma_start(out_core[t, :, HH:H], h_new[64:80, :])

                h_prev = h_new
                hT_cur = hT_new

    nc.compile()
    return nc


def _prep_core_inputs(x_c, Wd, Ud, bd, tc_n):
    """Host-side data prep for one core. x_c: [BL, t_steps, D] (already t-flipped
    for bwd cores)."""
    t_steps = tc_n * 8
    xp = np.ascontiguousarray(x_c.transpose(2, 1, 0))          # [D, T, BL]
    xp = xp.reshape(KT, 128, tc_n, 8, BL)
    x_prep = np.ascontiguousarray(xp.transpose(0, 2, 1, 3, 4)).reshape(KT, tc_n, 128, 128)

    Wp = Wd[:, _PERM]
    Up = Ud[:, _PERM]
    w_all = np.ascontiguousarray(Wp.reshape(KT, 128, G3))
    u_all = np.ascontiguousarray(Up.reshape(KT, 128, G3))

    b0p = bd[0][_PERM]
    b1p = bd[1][_PERM]
    zr = _ZR_MASK[_PERM]
    bfold_vec = (b0p + b1p * zr).astype(np.float32)
    bfold = np.broadcast_to(bfold_vec, (128, G3)).copy()

    b1h = np.empty((128, HH), np.float32)
    b1h[0:64] = b1p[768:1152]     # hA bias (features 0:384)
    b1h[64:128] = b1p[1920:2304]  # hB bias (features 384:768)

    ident = np.zeros((128, 16), np.float32)
    ident[0:16, 0:16] = np.eye(16)
    ident[64:80, 0:16] = np.eye(16)

    import ml_dtypes
    return {
        "x_prep": x_prep.astype(np.float32),
        "w_all": w_all.astype(np.float32),
        "u_all": u_all.astype(ml_dtypes.bfloat16),
        "bfold": bfold,
        "b1h": b1h,
        "ident": ident,
    }


_NC_CACHE = {}
LAST_RESULT = None


def kernel(inputs, W_fwd, U_fwd, b_fwd, W_bwd, U_bwd, b_bwd, training=0):
    inputs = np.asarray(inputs, np.float32)
    t_steps = inputs.shape[1]
    tc_n = t_steps // 8
    if t_steps not in _NC_CACHE:
        _NC_CACHE[t_steps] = build_program(t_steps)
    nc = _NC_CACHE[t_steps]

    in_maps = []
    for c in range(NCORE):
        dirn = c // 4
        bs = slice(BL * (c % 4), BL * (c % 4) + BL)
        x_c = inputs[bs]
        if dirn:
            x_c = x_c[:, ::-1]
        Wd, Ud, bd = (W_fwd, U_fwd, b_fwd) if dirn == 0 else (W_bwd, U_bwd, b_bwd)
        in_maps.append(_prep_core_inputs(x_c, np.asarray(Wd, np.float32),
                                         np.asarray(Ud, np.float32),
                                         np.asarray(bd, np.float32), tc_n))

    trace = bool(os.environ.get("GRU_TRACE"))
    kw = {}
    if trace:
        kw = dict(trace=True, tmpdir=os.environ.get("GRU_TRACE_DIR", "/tmp/gru_trace"))
    res = run_bass_kernel_spmd(nc, in_maps, list(range(NCORE)), **kw)
    global LAST_RESULT
    LAST_RESULT = res
    if res.exec_time_ns is not None:
        print(f"HW exec time: {res.exec_time_ns} ns")

    out = np.empty((B, t_steps, 2 * H), np.float32)
    for c in range(NCORE):
        dirn = c // 4
        bs = slice(BL * (c % 4), BL * (c % 4) + BL)
        oc = res.results[c]["out_core"]          # [T, BL, H]
        if dirn:
            oc = oc[::-1]
        out[bs, :, H * dirn:H * dirn + H] = oc.transpose(1, 0, 2)
    return out

